# revision 16
# baseline (speedup 1.0000x reference)
"""Trainium2 Bass kernel for nn_NeuronTurboModel_40819369181759 (nms_detection).

Computes, for N=8192 points on a 3D curve:
    total = sum(puffer(p2d)) + sum(cdist(p3d,p3d))/1e4
          + sum(relu(min_dist - masked_dist)) + 10*sum(second_diff(p2d)^2)
where masked_dist overwrites, per row i, the contiguous run of entries
j >= i with dist <= min_dist (prefix run from the diagonal).

Sharding: rows of the [N,N] distance matrix are split across 8 cores
(1024 rows each); the point set is replicated. Host combines per-core
per-partition partial sums.

Fast path (build_nc_fast):
  - d2 via one K=15 bf16 matmul per chunk: fp32 operands are split hi/lo
    into bf16 (x = h + l); d2 = Ah.Bh + Ah.Bl + Al.Bh stacked into K=15
    (bf16 products are exact; |dropped l.l| ~ 1e-7). PE runs at full bf16
    rate instead of the 4x fp32 penalty.
  - full row [128, 8192]: clamp d2 (relu; matmul cancellation noise can be
    ~ -1e-4 and HW sqrt(neg) = NaN), sqrt -> dist bf16 (accum -> SD).
    SR = sum relu(1-dist) is recovered without an extra pass from
    SR = rows*N - sum(min(dist, 1)) -- one DVE tensor_scalar with accum.
  - the prefix-run mask only needs a diagonal band: run lengths are
    bounded by the band width for curve-like inputs. Each 128-row tile
    processes a [128, 1024] slab starting at its first row (host-sliced
    per core, so SPMD-safe; columns past N are padded with a far point
    that breaks every run). In-slab j<i and j==i masks are compile-time
    static (u<p / u==p), host-provided.
  - per-row-tile slab work: its own small matmul -> clamp -> sqrt -> r;
    cond = (d2 <= 1) | (u < p); run = prefix-product scan; partial sums
    SRRS = sum r*run, BLK = sum r[:, 0:128] (the symmetric diagonal
    block), TR = sum diag(r). Host:
      masked = SRRS - (BLK - TR)/2
      total = SMALL + SD/1e4 + SR - masked
  - validity: if any run reaches the slab end (run[:, -1] != 0), the band
    assumption failed; host falls back to the exact full-width program
    (build_nc_full, the v1 kernel) -- correct for arbitrary inputs.
"""

import numpy as np

N_TOTAL = 8192
N_CORES = 8
ROWS_PER_CORE = N_TOTAL // N_CORES          # 1024
ROW_TILES = ROWS_PER_CORE // 128            # 8
SLAB = 1024                                 # band width (2 PSUM banks)
MIN_DIST = 1.0                              # RADIUS(0.5) * 2 * scale(1.0)

_CACHE = {}


def _split_excess_waits(nc, max_waits=1):
    """Split >max_waits sem waits per instruction onto preceding same-engine NOPs.

    The bundled walrus rejects instructions with several sync waits
    ("Too many sync wait commands"), while Tile's wait assignment can put one
    wait per producer proc on a single instruction. Waits execute in stream
    order on the owning engine, so hoisting the excess onto NOPs immediately
    before the instruction is semantics-preserving.
    """
    import concourse.mybir as mybir

    ctr = [0]
    for f in nc.m.functions:
        for b in f.blocks:
            changed = False
            new = []
            for inst in b.instructions:
                si = inst.sync_info
                if si is not None and len(si.on_wait) > max_waits:
                    waits = list(si.on_wait)
                    ups = list(si.on_update)
                    chunks = [
                        waits[i : i + max_waits]
                        for i in range(0, len(waits), max_waits)
                    ]
                    for ch in chunks[:-1]:
                        ctr[0] += 1
                        nop = mybir.InstNoOp(
                            name=f"I-waitsplit-{ctr[0]}", ins=[], outs=[]
                        )
                        nop.engine = inst.engine
                        nop.sync_info = mybir.SyncInfo(on_wait=ch, on_update=[])
                        new.append(nop)
                    inst.sync_info = mybir.SyncInfo(
                        on_wait=chunks[-1], on_update=ups
                    )
                    changed = True
                new.append(inst)
            if changed:
                b.instructions = new


def _small_losses(nc, pool, mybir, out_col, S):
    """Puffer + curvature losses on [128, S] layouts; ~1% of runtime."""
    f32 = mybir.dt.float32
    AF = mybir.ActivationFunctionType
    ALU = mybir.AluOpType
    AX = mybir.AxisListType

    y3_d = nc.dram_tensor("y3", [128, S + 2], f32, kind="ExternalInput")
    z3_d = nc.dram_tensor("z3", [128, S + 2], f32, kind="ExternalInput")
    y4_d = nc.dram_tensor("y4", [128, S], f32, kind="ExternalInput")
    z4_d = nc.dram_tensor("z4", [128, S], f32, kind="ExternalInput")
    flag_d = nc.dram_tensor("flag", [128, 1], f32, kind="ExternalInput")

    flag = pool.tile([128, 1], f32)
    nc.sync.dma_start(out=flag[:], in_=flag_d[:])
    y3 = pool.tile([128, S + 2], f32)
    z3 = pool.tile([128, S + 2], f32)
    y4 = pool.tile([128, S], f32)
    z4 = pool.tile([128, S], f32)
    nc.sync.dma_start(out=y3[:], in_=y3_d[:])
    nc.sync.dma_start(out=z3[:], in_=z3_d[:])
    nc.sync.dma_start(out=y4[:], in_=y4_d[:])
    nc.sync.dma_start(out=z4[:], in_=z4_d[:])

    fs = pool.tile([128, 4], f32)
    ns = pool.tile([128, 2], f32)
    scr = pool.tile([128, S], f32)
    t1 = pool.tile([128, S], f32)
    neg1 = pool.tile([128, 1], f32)
    nc.vector.memset(neg1[:], -1.0)

    # fixed (puffer): relu(v - 1) + relu(-1 - v) for v in {y, z}
    nc.scalar.activation(scr[:], y4[:], AF.Relu, bias=neg1[:], scale=1.0,
                         accum_out=fs[:, 0:1])
    nc.scalar.activation(scr[:], y4[:], AF.Relu, bias=neg1[:], scale=-1.0,
                         accum_out=fs[:, 1:2])
    nc.scalar.activation(scr[:], z4[:], AF.Relu, bias=neg1[:], scale=1.0,
                         accum_out=fs[:, 2:3])
    nc.scalar.activation(scr[:], z4[:], AF.Relu, bias=neg1[:], scale=-1.0,
                         accum_out=fs[:, 3:4])

    # noise: sum (v[k] - 2 v[k+1] + v[k+2])^2; host pads v3 by linear
    # extrapolation so the two out-of-range terms are exactly zero
    for i, v3 in enumerate((y3, z3)):
        nc.vector.scalar_tensor_tensor(
            out=t1[:], in0=v3[:, 1 : S + 1], scalar=-2.0, in1=v3[:, 0:S],
            op0=ALU.mult, op1=ALU.add,
        )
        nc.vector.tensor_tensor(t1[:], t1[:], v3[:, 2 : S + 2], ALU.add)
        nc.scalar.activation(scr[:], t1[:], AF.Square,
                             accum_out=ns[:, i : i + 1])

    fsum = pool.tile([128, 1], f32)
    nsum = pool.tile([128, 1], f32)
    nc.vector.tensor_reduce(out=fsum[:], in_=fs[:], axis=AX.X, op=ALU.add)
    nc.vector.tensor_reduce(out=nsum[:], in_=ns[:], axis=AX.X, op=ALU.add)
    nc.vector.scalar_tensor_tensor(
        out=fsum[:], in0=nsum[:], scalar=10.0, in1=fsum[:],
        op0=ALU.mult, op1=ALU.add,
    )
    nc.vector.tensor_tensor(out_col, fsum[:], flag[:], ALU.mult)


def build_nc_fast(n=N_TOTAL, row_tiles=ROW_TILES, slab=SLAB):
    """Band-limited fast program (same SPMD program on all 8 cores)."""
    import concourse.bass as bass
    import concourse.mybir as mybir
    from concourse.tile import TileContext

    f32 = mybir.dt.float32
    bf16 = mybir.dt.bfloat16
    AF = mybir.ActivationFunctionType
    ALU = mybir.AluOpType
    AX = mybir.AxisListType

    T = row_tiles
    S = n // 128
    md2 = MIN_DIST * MIN_DIST
    PW = 1024                       # psum tile width (2 banks)
    NQ = n // PW                    # full-row psum tiles per row tile

    nc = bass.Bass()

    aT_d = nc.dram_tensor("aT15", [T, 15, 128], bf16, kind="ExternalInput")
    bT_d = nc.dram_tensor("bT15", [15, n], bf16, kind="ExternalInput")
    bs_d = nc.dram_tensor("bTs", [T, 15, slab], bf16, kind="ExternalInput")
    lt_d = nc.dram_tensor("ltmask", [128, slab], bf16, kind="ExternalInput")
    id_d = nc.dram_tensor("ident", [128, 128], bf16, kind="ExternalInput")
    out_d = nc.dram_tensor("out", [128, 8], f32, kind="ExternalOutput")

    with TileContext(nc) as tc:
        with (
            tc.tile_pool(name="big", bufs=1) as big,
            tc.tile_pool(name="slb", bufs=2) as slb,
            tc.tile_pool(name="cst", bufs=1) as cst,
            tc.tile_pool(name="acc", bufs=1) as acc,
            tc.tile_pool(name="wts", bufs=2) as wts,
            tc.tile_pool(name="pf", bufs=2, space="PSUM") as pf,
            tc.tile_pool(name="psl", bufs=2, space="PSUM") as psl,
        ):
            bT = cst.tile([15, n], bf16)
            nc.sync.dma_start(out=bT[:], in_=bT_d[:])
            ltm = cst.tile([128, slab], bf16)
            nc.sync.dma_start(out=ltm[:], in_=lt_d[:])
            ident = cst.tile([128, 128], bf16)
            nc.sync.dma_start(out=ident[:], in_=id_d[:])

            sd_slots = acc.tile([128, T], f32)
            srm_slots = acc.tile([128, T], f32)
            srr_slots = acc.tile([128, T], f32)
            blk_slots = acc.tile([128, T], f32)
            tr_slots = acc.tile([128, T], f32)
            re_slots = acc.tile([128, T], f32)
            out_sb = acc.tile([128, 8], f32)

            _small_losses(nc, acc, mybir, out_sb[:, 5:6], S)

            for t in range(T):
                aTt = wts.tile([15, 128], bf16, tag="aTt")
                nc.sync.dma_start(out=aTt[:], in_=aT_d[t])
                bst = wts.tile([15, slab], bf16, tag="bst")
                nc.sync.dma_start(out=bst[:], in_=bs_d[t])

                # ---- full row: d2 -> dist -> SD accum; min(dist,1) -> SR.
                # d2/dist stay f32: SR = rows*n - SRMIN needs SRMIN accurate
                # to ~5e-6 relative, which bf16-rounded d2/dist break.
                dist = big.tile([128, n], f32, tag="dist")
                scr = big.tile([128, n], f32, tag="scr")
                sink = big.tile([128, n], bf16, tag="sink")
                for q in range(NQ):
                    pk = pf.tile([128, PW], f32, tag="pk")
                    for h in range(PW // 512):
                        c0 = q * PW + h * 512
                        nc.tensor.matmul(
                            pk[:, h * 512 : (h + 1) * 512],
                            aTt[:], bT[:, c0 : c0 + 512],
                            start=True, stop=True,
                        )
                    # clamp cancellation noise; split across ACT/DVE
                    if (t + q) % 2 == 0:
                        nc.scalar.activation(
                            scr[:, q * PW : (q + 1) * PW], pk[:], AF.Relu
                        )
                    else:
                        nc.vector.tensor_scalar(
                            out=scr[:, q * PW : (q + 1) * PW], in0=pk[:],
                            scalar1=0.0, scalar2=None, op0=ALU.max,
                        )
                nc.scalar.activation(dist[:], scr[:], AF.Sqrt,
                                     accum_out=sd_slots[:, t : t + 1])
                # SR = rows*n - sum(min(dist,1)); accum is f32-internal,
                # the bf16 sink write is discarded (bf16 out keeps DVE 2x)
                nc.vector.tensor_scalar(
                    out=sink[:], in0=dist[:], scalar1=1.0, scalar2=0.0,
                    op0=ALU.min, op1=ALU.add,
                    accum_out=srm_slots[:, t : t + 1],
                )

                # ---- slab: band-limited prefix-run mask ----
                ps = psl.tile([128, slab], f32, tag="ps")
                for h in range(slab // 512):
                    nc.tensor.matmul(
                        ps[:, h * 512 : (h + 1) * 512],
                        aTt[:], bst[:, h * 512 : (h + 1) * 512],
                        start=True, stop=True,
                    )
                d2s = slb.tile([128, slab], f32, tag="d2s")
                nc.scalar.activation(d2s[:], ps[:], AF.Relu)
                dss = slb.tile([128, slab], f32, tag="dss")
                nc.scalar.activation(dss[:], d2s[:], AF.Sqrt)
                rs = slb.tile([128, slab], f32, tag="rs")
                nc.scalar.activation(rs[:], dss[:], AF.Relu,
                                     bias=1.0, scale=-1.0)

                cond = slb.tile([128, slab], bf16, tag="cond")
                nc.vector.scalar_tensor_tensor(
                    out=cond[:], in0=d2s[:], scalar=md2, in1=ltm[:],
                    op0=ALU.is_le, op1=ALU.logical_or,
                )
                run = slb.tile([128, slab], bf16, tag="run")
                nc.vector.tensor_tensor_scan(
                    out=run[:], data0=cond[:], data1=cond[:], initial=1.0,
                    op0=ALU.mult, op1=ALU.min,
                )
                nc.vector.scalar_tensor_tensor(
                    out=cond[:], in0=rs[:], scalar=1.0, in1=run[:],
                    op0=ALU.mult, op1=ALU.mult,
                    accum_out=srr_slots[:, t : t + 1],
                )
                nc.vector.tensor_scalar(
                    out=cond[:, 0:128], in0=rs[:, 0:128], scalar1=1.0,
                    scalar2=0.0, op0=ALU.mult, op1=ALU.add,
                    accum_out=blk_slots[:, t : t + 1],
                )
                nc.vector.scalar_tensor_tensor(
                    out=cond[:, 0:128], in0=rs[:, 0:128], scalar=1.0,
                    in1=ident[:], op0=ALU.mult, op1=ALU.mult,
                    accum_out=tr_slots[:, t : t + 1],
                )
                nc.vector.tensor_copy(
                    re_slots[:, t : t + 1], run[:, slab - 1 : slab]
                )

            nc.vector.tensor_reduce(out=out_sb[:, 0:1], in_=sd_slots[:],
                                    axis=AX.X, op=ALU.add)
            nc.vector.tensor_reduce(out=out_sb[:, 1:2], in_=srm_slots[:],
                                    axis=AX.X, op=ALU.add)
            nc.vector.tensor_reduce(out=out_sb[:, 2:3], in_=srr_slots[:],
                                    axis=AX.X, op=ALU.add)
            nc.vector.tensor_reduce(out=out_sb[:, 3:4], in_=blk_slots[:],
                                    axis=AX.X, op=ALU.add)
            nc.vector.tensor_reduce(out=out_sb[:, 4:5], in_=tr_slots[:],
                                    axis=AX.X, op=ALU.add)
            nc.vector.tensor_reduce(out=out_sb[:, 6:7], in_=re_slots[:],
                                    axis=AX.X, op=ALU.max)
            nc.vector.memset(out_sb[:, 7:8], 0.0)
            nc.sync.dma_start(out=out_d[:], in_=out_sb[:])

    return nc


def _bf16_split(a):
    import ml_dtypes
    bf = ml_dtypes.bfloat16
    h = a.astype(bf)
    l = (a - h.astype(np.float32)).astype(bf)
    return h, l


def make_core_inputs_fast(x, y, z, n=N_TOTAL, n_cores=N_CORES, slab=SLAB):
    import ml_dtypes
    bf = ml_dtypes.bfloat16
    x = np.asarray(x, np.float32)
    y = np.asarray(y, np.float32)
    z = np.asarray(z, np.float32)
    rpc = n // n_cores
    T = rpc // 128
    S = n // 128

    sq = x * x + y * y + z * z
    ones = np.ones(n, np.float32)
    A = np.stack([x, y, z, sq, ones], 1)             # [n, 5]
    B = np.stack([-2 * x, -2 * y, -2 * z, ones, sq], 1)
    Ah, Al = _bf16_split(A)
    Bh, Bl = _bf16_split(B)
    A15 = np.concatenate([Ah, Ah, Al], 1)            # [n, 15] bf16
    B15 = np.concatenate([Bh, Bl, Bh], 1)
    bT15 = np.ascontiguousarray(B15.T)               # [15, n]

    # pad columns past n with a far point (breaks runs, contributes r=0)
    far = np.array([1e3, 0.0, 0.0], np.float32)
    fsq = np.float32(far @ far)
    Bf = np.array([-2 * far[0], -2 * far[1], -2 * far[2], 1.0, fsq],
                  np.float32)
    Bfh, Bfl = _bf16_split(Bf)
    Bf15 = np.concatenate([Bfh, Bfl, Bfh]).astype(bf)  # [15]
    B15p = np.concatenate([B15, np.broadcast_to(Bf15, (slab, 15))], 0)

    ltmask = (np.arange(slab)[None, :] < np.arange(128)[:, None]).astype(bf)
    ident = np.eye(128).astype(bf)

    def _pad_extrap(v):
        two = np.float32(2.0)
        p0 = np.float32(two * v[-1] - v[-2])
        p1 = -np.float32(v[-1] - two * p0)
        return np.concatenate([v, [p0, p1]]).astype(np.float32)

    ypad = _pad_extrap(y)
    zpad = _pad_extrap(z)
    idx3 = S * np.arange(128)[:, None] + np.arange(S + 2)[None, :]
    y3 = ypad[idx3].astype(np.float32)
    z3 = zpad[idx3].astype(np.float32)
    y4 = y.reshape(128, S).copy()
    z4 = z.reshape(128, S).copy()

    in_maps = []
    for c in range(n_cores):
        rows = slice(c * rpc, (c + 1) * rpc)
        aT15 = np.ascontiguousarray(
            A15[rows].reshape(T, 128, 15).transpose(0, 2, 1)
        )                                            # [T, 15, 128]
        bTs = np.stack([
            np.ascontiguousarray(B15p[c * rpc + 128 * t:
                                      c * rpc + 128 * t + slab].T)
            for t in range(T)
        ])                                           # [T, 15, slab]
        flag = np.full((128, 1), 1.0 if c == 0 else 0.0, np.float32)
        in_maps.append({
            "aT15": aT15, "bT15": bT15, "bTs": bTs,
            "ltmask": ltmask, "ident": ident,
            "y3": y3, "z3": z3, "y4": y4, "z4": z4, "flag": flag,
        })
    return in_maps


def combine_fast(outs, n=N_TOTAL):
    """Returns (total, band_overflowed)."""
    s = np.zeros(8, np.float64)
    overflow = 0.0
    for o in outs:
        a = np.asarray(o, np.float64)
        s += a.sum(axis=0)
        overflow = max(overflow, a[:, 6].max())
    SD, SRMIN, SRRS, BLK, TR, SMALL = s[0], s[1], s[2], s[3], s[4], s[5]
    SR = float(n) * float(n) - SRMIN
    masked = SRRS - (BLK - TR) / 2.0
    total = SMALL + SD / 1e4 + SR - masked
    return np.array(total, dtype=np.float32), overflow > 0.0


# ---------------------------------------------------------------------------
# Fallback: exact full-width program for inputs whose prefix runs exceed
# the band. Full-row scan; lower-triangle sums via symmetry of r.
#   intersect = SR - SRR + (SR - N)/2
# ---------------------------------------------------------------------------

def build_nc_full(n=N_TOTAL, row_tiles=ROW_TILES, chunk=512):
    import concourse.bass as bass
    import concourse.mybir as mybir
    from concourse.tile import TileContext

    f32 = mybir.dt.float32
    bf16 = mybir.dt.bfloat16
    AF = mybir.ActivationFunctionType
    ALU = mybir.AluOpType
    AX = mybir.AxisListType

    T = row_tiles
    n_chunks = n // chunk
    OFF = 128 * (T - 1)
    W = n + OFF
    md2 = MIN_DIST * MIN_DIST
    S = n // 128

    nc = bass.Bass()

    aT_d = nc.dram_tensor("aT", [T, 5, 128], f32, kind="ExternalInput")
    bT_d = nc.dram_tensor("bT", [5, n], f32, kind="ExternalInput")
    rowv0_d = nc.dram_tensor("rowv0", [128, 1], f32, kind="ExternalInput")
    out_d = nc.dram_tensor("out", [128, 4], f32, kind="ExternalOutput")

    with TileContext(nc) as tc:
        with (
            tc.tile_pool(name="big", bufs=1) as big,
            tc.tile_pool(name="cst", bufs=1) as cst,
            tc.tile_pool(name="acc", bufs=1) as acc,
            tc.tile_pool(name="wts", bufs=2) as wts,
            tc.tile_pool(name="psum", bufs=8, space="PSUM") as psum,
        ):
            bT = cst.tile([5, n], f32)
            nc.sync.dma_start(out=bT[:], in_=bT_d[:])
            rowv0 = cst.tile([128, 1], f32)
            nc.sync.dma_start(out=rowv0[:], in_=rowv0_d[:])

            iotaM = big.tile([128, W], f32, tag="r")
            nc.gpsimd.iota(
                iotaM[:], pattern=[[1, W]], base=0, channel_multiplier=0,
                allow_small_or_imprecise_dtypes=True,
            )
            ltM = cst.tile([128, W], bf16)
            nc.gpsimd.tensor_scalar(
                out=ltM[:], in0=iotaM[:], scalar1=float(OFF),
                scalar2=rowv0[:], op0=ALU.subtract, op1=ALU.is_lt,
            )

            sd_slots = acc.tile([128, T], f32)
            sr_slots = acc.tile([128, T], f32)
            srr_slots = acc.tile([128, T], f32)
            out_sb = acc.tile([128, 4], f32)

            _small_losses(nc, acc, mybir, out_sb[:, 3:4], S)

            for t in range(T):
                aT_t = wts.tile([5, 128], f32, tag="aT")
                nc.sync.dma_start(out=aT_t[:], in_=aT_d[t])

                d2c = big.tile([128, n], f32, tag="d2c")
                for k in range(n_chunks):
                    pk = psum.tile([128, chunk], f32, tag="pk")
                    nc.tensor.matmul(
                        pk[:], aT_t[:], bT[:, k * chunk : (k + 1) * chunk],
                        start=True, stop=True,
                    )
                    nc.scalar.activation(
                        d2c[:, k * chunk : (k + 1) * chunk], pk[:], AF.Relu
                    )

                dist = big.tile([128, n], f32, tag="dist")
                nc.scalar.activation(dist[:], d2c[:], AF.Sqrt,
                                     accum_out=sd_slots[:, t : t + 1])
                r = big.tile([128, n], f32, tag="r")
                nc.scalar.activation(r[:], dist[:], AF.Relu, bias=1.0,
                                     scale=-1.0,
                                     accum_out=sr_slots[:, t : t + 1])

                cond = big.tile([128, n], bf16, tag="cond")
                s0 = OFF - 128 * t
                nc.vector.scalar_tensor_tensor(
                    out=cond[:], in0=d2c[:], scalar=md2,
                    in1=ltM[:, s0 : s0 + n],
                    op0=ALU.is_le, op1=ALU.logical_or,
                )
                run = big.tile([128, n], bf16, tag="run")
                nc.vector.tensor_tensor_scan(
                    out=run[:], data0=cond[:], data1=cond[:], initial=1.0,
                    op0=ALU.mult, op1=ALU.min,
                )
                nc.vector.scalar_tensor_tensor(
                    out=cond[:], in0=r[:], scalar=1.0, in1=run[:],
                    op0=ALU.mult, op1=ALU.mult,
                    accum_out=srr_slots[:, t : t + 1],
                )

            nc.vector.tensor_reduce(out=out_sb[:, 0:1], in_=sd_slots[:],
                                    axis=AX.X, op=ALU.add)
            nc.vector.tensor_reduce(out=out_sb[:, 1:2], in_=sr_slots[:],
                                    axis=AX.X, op=ALU.add)
            nc.vector.tensor_reduce(out=out_sb[:, 2:3], in_=srr_slots[:],
                                    axis=AX.X, op=ALU.add)
            nc.sync.dma_start(out=out_d[:], in_=out_sb[:])

    return nc


def make_core_inputs_full(x, y, z, n=N_TOTAL, n_cores=N_CORES):
    x = np.asarray(x, np.float32)
    y = np.asarray(y, np.float32)
    z = np.asarray(z, np.float32)
    rpc = n // n_cores
    T = rpc // 128
    S = n // 128

    sq = x * x + y * y + z * z
    ones = np.ones(n, np.float32)
    B = np.stack([-2.0 * x, -2.0 * y, -2.0 * z, ones, sq]).astype(np.float32)

    def _pad_extrap(v):
        two = np.float32(2.0)
        p0 = np.float32(two * v[-1] - v[-2])
        p1 = -np.float32(v[-1] - two * p0)
        return np.concatenate([v, [p0, p1]]).astype(np.float32)

    ypad = _pad_extrap(y)
    zpad = _pad_extrap(z)
    idx3 = S * np.arange(128)[:, None] + np.arange(S + 2)[None, :]
    y3 = ypad[idx3].astype(np.float32)
    z3 = zpad[idx3].astype(np.float32)
    y4 = y.reshape(128, S).copy()
    z4 = z.reshape(128, S).copy()

    in_maps = []
    for c in range(n_cores):
        sl = slice(c * rpc, (c + 1) * rpc)
        A = np.stack([x[sl], y[sl], z[sl], sq[sl], ones[sl]], axis=1)
        aT = np.ascontiguousarray(
            A.reshape(T, 128, 5).transpose(0, 2, 1)
        ).astype(np.float32)
        rowv0 = (c * rpc + np.arange(128, dtype=np.float32)).reshape(128, 1)
        flag = np.full((128, 1), 1.0 if c == 0 else 0.0, np.float32)
        in_maps.append({
            "aT": aT, "bT": B, "rowv0": rowv0,
            "y3": y3, "z3": z3, "y4": y4, "z4": z4, "flag": flag,
        })
    return in_maps


def combine_full(outs, n=N_TOTAL):
    s = np.zeros(4, np.float64)
    for o in outs:
        s += np.asarray(o, np.float64).sum(axis=0)
    SD, SR, SRR, SMALL = s
    intersect = SR - SRR + (SR - float(n)) / 2.0
    total = SMALL + SD / 1e4 + intersect
    return np.array(total, dtype=np.float32)


def kernel(x, y, z):
    from concourse.bass_utils import run_bass_kernel_spmd

    if "fast" not in _CACHE:
        nc = build_nc_fast()
        _split_excess_waits(nc)
        _CACHE["fast"] = nc

    in_maps = make_core_inputs_fast(x, y, z)
    res = run_bass_kernel_spmd(_CACHE["fast"], in_maps, list(range(N_CORES)))
    outs = [res.results[c]["out"] for c in range(N_CORES)]
    total, overflow = combine_fast(outs)
    if not overflow:
        return total

    # a prefix run reached the band edge: rerun with the exact full-width
    # program (arbitrary-input correct, slower)
    if "full" not in _CACHE:
        nc = build_nc_full()
        _split_excess_waits(nc)
        _CACHE["full"] = nc
    in_maps = make_core_inputs_full(x, y, z)
    res = run_bass_kernel_spmd(_CACHE["full"], in_maps, list(range(N_CORES)))
    outs = [res.results[c]["out"] for c in range(N_CORES)]
    return combine_full(outs)


# revision 17
# speedup vs baseline: 1.1792x; 1.1792x over previous
"""Trainium2 Bass kernel for nn_NeuronTurboModel_40819369181759 (nms_detection).

Computes, for N=8192 points on a 3D curve:
    total = sum(puffer(p2d)) + sum(cdist(p3d,p3d))/1e4
          + sum(relu(min_dist - masked_dist)) + 10*sum(second_diff(p2d)^2)
where masked_dist overwrites, per row i, the contiguous run of entries
j >= i with dist <= min_dist (prefix run from the diagonal).

Sharding: rows of the [N,N] distance matrix are split across 8 cores
(1024 rows each); the point set is replicated. Host combines per-core
per-partition partial sums.

Fast path (build_nc_fast):
  - d2 via one K=15 bf16 matmul per chunk: fp32 operands are split hi/lo
    into bf16 (x = h + l); d2 = Ah.Bh + Ah.Bl + Al.Bh stacked into K=15
    (bf16 products are exact; |dropped l.l| ~ 1e-7). PE runs at full bf16
    rate instead of the 4x fp32 penalty.
  - full row [128, 8192]: clamp d2 (relu; matmul cancellation noise can be
    ~ -1e-4 and HW sqrt(neg) = NaN), sqrt -> dist bf16 (accum -> SD).
    SR = sum relu(1-dist) is recovered without an extra pass from
    SR = rows*N - sum(min(dist, 1)) -- one DVE tensor_scalar with accum.
  - the prefix-run mask only needs a diagonal band: run lengths are
    bounded by the band width for curve-like inputs. Each 128-row tile
    processes a [128, 1024] slab starting at its first row (host-sliced
    per core, so SPMD-safe; columns past N are padded with a far point
    that breaks every run). In-slab j<i and j==i masks are compile-time
    static (u<p / u==p), host-provided.
  - per-row-tile slab work: its own small matmul -> clamp -> sqrt -> r;
    cond = (d2 <= 1) | (u < p); run = prefix-product scan; partial sums
    SRRS = sum r*run, BLK = sum r[:, 0:128] (the symmetric diagonal
    block), TR = sum diag(r). Host:
      masked = SRRS - (BLK - TR)/2
      total = SMALL + SD/1e4 + SR - masked
  - validity: if any run reaches the slab end (run[:, -1] != 0), the band
    assumption failed; host falls back to the exact full-width program
    (build_nc_full, the v1 kernel) -- correct for arbitrary inputs.
"""

import numpy as np

N_TOTAL = 8192
N_CORES = 8
ROWS_PER_CORE = N_TOTAL // N_CORES          # 1024
ROW_TILES = ROWS_PER_CORE // 128            # 8
SLAB = 1024                                 # band width (2 PSUM banks)
MIN_DIST = 1.0                              # RADIUS(0.5) * 2 * scale(1.0)

_CACHE = {}


def _split_excess_waits(nc, max_waits=1):
    """Split >max_waits sem waits per instruction onto preceding same-engine NOPs.

    The bundled walrus rejects instructions with several sync waits
    ("Too many sync wait commands"), while Tile's wait assignment can put one
    wait per producer proc on a single instruction. Waits execute in stream
    order on the owning engine, so hoisting the excess onto NOPs immediately
    before the instruction is semantics-preserving.
    """
    import concourse.mybir as mybir

    ctr = [0]
    for f in nc.m.functions:
        for b in f.blocks:
            changed = False
            new = []
            for inst in b.instructions:
                si = inst.sync_info
                if si is not None and len(si.on_wait) > max_waits:
                    waits = list(si.on_wait)
                    ups = list(si.on_update)
                    chunks = [
                        waits[i : i + max_waits]
                        for i in range(0, len(waits), max_waits)
                    ]
                    for ch in chunks[:-1]:
                        ctr[0] += 1
                        nop = mybir.InstNoOp(
                            name=f"I-waitsplit-{ctr[0]}", ins=[], outs=[]
                        )
                        nop.engine = inst.engine
                        nop.sync_info = mybir.SyncInfo(on_wait=ch, on_update=[])
                        new.append(nop)
                    inst.sync_info = mybir.SyncInfo(
                        on_wait=chunks[-1], on_update=ups
                    )
                    changed = True
                new.append(inst)
            if changed:
                b.instructions = new


def _small_losses(nc, pool, mybir, out_col, S):
    """Puffer + curvature losses on [128, S] layouts; ~1% of runtime."""
    f32 = mybir.dt.float32
    AF = mybir.ActivationFunctionType
    ALU = mybir.AluOpType
    AX = mybir.AxisListType

    y3_d = nc.dram_tensor("y3", [128, S + 2], f32, kind="ExternalInput")
    z3_d = nc.dram_tensor("z3", [128, S + 2], f32, kind="ExternalInput")
    y4_d = nc.dram_tensor("y4", [128, S], f32, kind="ExternalInput")
    z4_d = nc.dram_tensor("z4", [128, S], f32, kind="ExternalInput")
    flag_d = nc.dram_tensor("flag", [128, 1], f32, kind="ExternalInput")

    flag = pool.tile([128, 1], f32)
    nc.sync.dma_start(out=flag[:], in_=flag_d[:])
    y3 = pool.tile([128, S + 2], f32)
    z3 = pool.tile([128, S + 2], f32)
    y4 = pool.tile([128, S], f32)
    z4 = pool.tile([128, S], f32)
    nc.sync.dma_start(out=y3[:], in_=y3_d[:])
    nc.sync.dma_start(out=z3[:], in_=z3_d[:])
    nc.sync.dma_start(out=y4[:], in_=y4_d[:])
    nc.sync.dma_start(out=z4[:], in_=z4_d[:])

    fs = pool.tile([128, 4], f32)
    ns = pool.tile([128, 2], f32)
    scr = pool.tile([128, S], f32)
    t1 = pool.tile([128, S], f32)
    neg1 = pool.tile([128, 1], f32)
    nc.vector.memset(neg1[:], -1.0)

    # fixed (puffer): relu(v - 1) + relu(-1 - v) for v in {y, z}
    nc.scalar.activation(scr[:], y4[:], AF.Relu, bias=neg1[:], scale=1.0,
                         accum_out=fs[:, 0:1])
    nc.scalar.activation(scr[:], y4[:], AF.Relu, bias=neg1[:], scale=-1.0,
                         accum_out=fs[:, 1:2])
    nc.scalar.activation(scr[:], z4[:], AF.Relu, bias=neg1[:], scale=1.0,
                         accum_out=fs[:, 2:3])
    nc.scalar.activation(scr[:], z4[:], AF.Relu, bias=neg1[:], scale=-1.0,
                         accum_out=fs[:, 3:4])

    # noise: sum (v[k] - 2 v[k+1] + v[k+2])^2; host pads v3 by linear
    # extrapolation so the two out-of-range terms are exactly zero
    for i, v3 in enumerate((y3, z3)):
        nc.vector.scalar_tensor_tensor(
            out=t1[:], in0=v3[:, 1 : S + 1], scalar=-2.0, in1=v3[:, 0:S],
            op0=ALU.mult, op1=ALU.add,
        )
        nc.vector.tensor_tensor(t1[:], t1[:], v3[:, 2 : S + 2], ALU.add)
        nc.scalar.activation(scr[:], t1[:], AF.Square,
                             accum_out=ns[:, i : i + 1])

    fsum = pool.tile([128, 1], f32)
    nsum = pool.tile([128, 1], f32)
    nc.vector.tensor_reduce(out=fsum[:], in_=fs[:], axis=AX.X, op=ALU.add)
    nc.vector.tensor_reduce(out=nsum[:], in_=ns[:], axis=AX.X, op=ALU.add)
    nc.vector.scalar_tensor_tensor(
        out=fsum[:], in0=nsum[:], scalar=10.0, in1=fsum[:],
        op0=ALU.mult, op1=ALU.add,
    )
    nc.vector.tensor_tensor(out_col, fsum[:], flag[:], ALU.mult)


def build_nc_fast(n=N_TOTAL, row_tiles=ROW_TILES, slab=SLAB):
    """Band-limited fast program (same SPMD program on all 8 cores)."""
    import concourse.bass as bass
    import concourse.mybir as mybir
    from concourse.tile import TileContext

    f32 = mybir.dt.float32
    bf16 = mybir.dt.bfloat16
    AF = mybir.ActivationFunctionType
    ALU = mybir.AluOpType
    AX = mybir.AxisListType

    T = row_tiles
    S = n // 128
    md2 = MIN_DIST * MIN_DIST
    PW = 1024                       # psum tile width (2 banks)
    NQ = n // PW                    # full-row psum tiles per row tile

    nc = bass.Bass()

    aT_d = nc.dram_tensor("aT15", [T, 15, 128], bf16, kind="ExternalInput")
    bT_d = nc.dram_tensor("bT15", [15, n], bf16, kind="ExternalInput")
    bs_d = nc.dram_tensor("bTs", [T, 15, slab], bf16, kind="ExternalInput")
    lt_d = nc.dram_tensor("ltmask", [128, slab], bf16, kind="ExternalInput")
    id_d = nc.dram_tensor("ident", [128, 128], bf16, kind="ExternalInput")
    out_d = nc.dram_tensor("out", [128, 8], f32, kind="ExternalOutput")

    with TileContext(nc) as tc:
        with (
            tc.tile_pool(name="big", bufs=1) as big,
            tc.tile_pool(name="slb", bufs=2) as slb,
            tc.tile_pool(name="cst", bufs=1) as cst,
            tc.tile_pool(name="acc", bufs=1) as acc,
            tc.tile_pool(name="wts", bufs=2) as wts,
            tc.tile_pool(name="pf", bufs=2, space="PSUM") as pf,
            tc.tile_pool(name="psl", bufs=2, space="PSUM") as psl,
        ):
            bT = cst.tile([15, n], bf16)
            nc.sync.dma_start(out=bT[:], in_=bT_d[:])
            ltm = cst.tile([128, slab], bf16)
            nc.sync.dma_start(out=ltm[:], in_=lt_d[:])
            ident = cst.tile([128, 128], bf16)
            nc.sync.dma_start(out=ident[:], in_=id_d[:])

            sd_slots = acc.tile([128, T], f32)
            srm_slots = acc.tile([128, T], f32)
            srr_slots = acc.tile([128, T], f32)
            blk_slots = acc.tile([128, T], f32)
            tr_slots = acc.tile([128, T], f32)
            re_slots = acc.tile([128, T], f32)
            out_sb = acc.tile([128, 8], f32)

            _small_losses(nc, acc, mybir, out_sb[:, 5:6], S)

            for t in range(T):
                aTt = wts.tile([15, 128], bf16, tag="aTt")
                nc.sync.dma_start(out=aTt[:], in_=aT_d[t])
                bst = wts.tile([15, slab], bf16, tag="bst")
                nc.sync.dma_start(out=bst[:], in_=bs_d[t])

                # ---- full row: d2 -> dist -> SD accum; relu(1-dist) -> SR.
                # d2/dist stay f32: SR needs ~5e-6 relative accuracy, which
                # bf16-rounded d2/dist break.
                dist = big.tile([128, n], f32, tag="dist")
                scr = big.tile([128, n], f32, tag="scr")
                sink = big.tile([128, n], bf16, tag="sink")
                for q in range(NQ):
                    pk = pf.tile([128, PW], f32, tag="pk")
                    for h in range(PW // 512):
                        c0 = q * PW + h * 512
                        nc.tensor.matmul(
                            pk[:, h * 512 : (h + 1) * 512],
                            aTt[:], bT[:, c0 : c0 + 512],
                            start=True, stop=True,
                        )
                    # clamp cancellation noise; mostly on DVE (ACT is the
                    # busier engine: sqrt + r-pass)
                    if (t * NQ + q) % 7 == 3:
                        nc.scalar.activation(
                            scr[:, q * PW : (q + 1) * PW], pk[:], AF.Relu
                        )
                    else:
                        nc.vector.tensor_scalar(
                            out=scr[:, q * PW : (q + 1) * PW], in0=pk[:],
                            scalar1=0.0, scalar2=None, op0=ALU.max,
                        )
                nc.scalar.activation(dist[:], scr[:], AF.Sqrt,
                                     accum_out=sd_slots[:, t : t + 1])
                # SR partial: ACT relu(1 - dist) with accum; output discarded
                nc.scalar.activation(sink[:], dist[:], AF.Relu,
                                     bias=1.0, scale=-1.0,
                                     accum_out=srm_slots[:, t : t + 1])

                # ---- slab: band-limited prefix-run mask ----
                ps = psl.tile([128, slab], f32, tag="ps")
                for h in range(slab // 512):
                    nc.tensor.matmul(
                        ps[:, h * 512 : (h + 1) * 512],
                        aTt[:], bst[:, h * 512 : (h + 1) * 512],
                        start=True, stop=True,
                    )
                d2s = slb.tile([128, slab], f32, tag="d2s")
                nc.vector.tensor_scalar(out=d2s[:], in0=ps[:],
                                        scalar1=0.0, scalar2=None, op0=ALU.max)
                dss = slb.tile([128, slab], f32, tag="dss")
                nc.scalar.activation(dss[:], d2s[:], AF.Sqrt)
                # rs = -relu(1 - dist_s) = min(dist_s - 1, 0); the host
                # negates the SRRS/BLK/TR partials it feeds
                rs = slb.tile([128, slab], f32, tag="rs")
                nc.vector.tensor_scalar(out=rs[:], in0=dss[:],
                                        scalar1=1.0, scalar2=0.0,
                                        op0=ALU.subtract, op1=ALU.min)

                cond = slb.tile([128, slab], bf16, tag="cond")
                nc.vector.scalar_tensor_tensor(
                    out=cond[:], in0=d2s[:], scalar=md2, in1=ltm[:],
                    op0=ALU.is_le, op1=ALU.logical_or,
                )
                run = slb.tile([128, slab], bf16, tag="run")
                nc.vector.tensor_tensor_scan(
                    out=run[:], data0=cond[:], data1=cond[:], initial=1.0,
                    op0=ALU.mult, op1=ALU.min,
                )
                nc.vector.scalar_tensor_tensor(
                    out=cond[:], in0=rs[:], scalar=1.0, in1=run[:],
                    op0=ALU.mult, op1=ALU.mult,
                    accum_out=srr_slots[:, t : t + 1],
                )
                nc.vector.tensor_scalar(
                    out=cond[:, 0:128], in0=rs[:, 0:128], scalar1=1.0,
                    scalar2=0.0, op0=ALU.mult, op1=ALU.add,
                    accum_out=blk_slots[:, t : t + 1],
                )
                nc.vector.scalar_tensor_tensor(
                    out=cond[:, 0:128], in0=rs[:, 0:128], scalar=1.0,
                    in1=ident[:], op0=ALU.mult, op1=ALU.mult,
                    accum_out=tr_slots[:, t : t + 1],
                )
                nc.vector.tensor_copy(
                    re_slots[:, t : t + 1], run[:, slab - 1 : slab]
                )

            nc.vector.tensor_reduce(out=out_sb[:, 0:1], in_=sd_slots[:],
                                    axis=AX.X, op=ALU.add)
            nc.vector.tensor_reduce(out=out_sb[:, 1:2], in_=srm_slots[:],
                                    axis=AX.X, op=ALU.add)
            nc.vector.tensor_reduce(out=out_sb[:, 2:3], in_=srr_slots[:],
                                    axis=AX.X, op=ALU.add)
            nc.vector.tensor_reduce(out=out_sb[:, 3:4], in_=blk_slots[:],
                                    axis=AX.X, op=ALU.add)
            nc.vector.tensor_reduce(out=out_sb[:, 4:5], in_=tr_slots[:],
                                    axis=AX.X, op=ALU.add)
            nc.vector.tensor_reduce(out=out_sb[:, 6:7], in_=re_slots[:],
                                    axis=AX.X, op=ALU.max)
            nc.vector.memset(out_sb[:, 7:8], 0.0)
            nc.sync.dma_start(out=out_d[:], in_=out_sb[:])

    return nc


def _bf16_split(a):
    import ml_dtypes
    bf = ml_dtypes.bfloat16
    h = a.astype(bf)
    l = (a - h.astype(np.float32)).astype(bf)
    return h, l


def make_core_inputs_fast(x, y, z, n=N_TOTAL, n_cores=N_CORES, slab=SLAB):
    import ml_dtypes
    bf = ml_dtypes.bfloat16
    x = np.asarray(x, np.float32)
    y = np.asarray(y, np.float32)
    z = np.asarray(z, np.float32)
    rpc = n // n_cores
    T = rpc // 128
    S = n // 128

    sq = x * x + y * y + z * z
    ones = np.ones(n, np.float32)
    A = np.stack([x, y, z, sq, ones], 1)             # [n, 5]
    B = np.stack([-2 * x, -2 * y, -2 * z, ones, sq], 1)
    Ah, Al = _bf16_split(A)
    Bh, Bl = _bf16_split(B)
    A15 = np.concatenate([Ah, Ah, Al], 1)            # [n, 15] bf16
    B15 = np.concatenate([Bh, Bl, Bh], 1)
    bT15 = np.ascontiguousarray(B15.T)               # [15, n]

    # pad columns past n with a far point (breaks runs, contributes r=0)
    far = np.array([1e3, 0.0, 0.0], np.float32)
    fsq = np.float32(far @ far)
    Bf = np.array([-2 * far[0], -2 * far[1], -2 * far[2], 1.0, fsq],
                  np.float32)
    Bfh, Bfl = _bf16_split(Bf)
    Bf15 = np.concatenate([Bfh, Bfl, Bfh]).astype(bf)  # [15]
    B15p = np.concatenate([B15, np.broadcast_to(Bf15, (slab, 15))], 0)

    ltmask = (np.arange(slab)[None, :] < np.arange(128)[:, None]).astype(bf)
    ident = np.eye(128).astype(bf)

    def _pad_extrap(v):
        two = np.float32(2.0)
        p0 = np.float32(two * v[-1] - v[-2])
        p1 = -np.float32(v[-1] - two * p0)
        return np.concatenate([v, [p0, p1]]).astype(np.float32)

    ypad = _pad_extrap(y)
    zpad = _pad_extrap(z)
    idx3 = S * np.arange(128)[:, None] + np.arange(S + 2)[None, :]
    y3 = ypad[idx3].astype(np.float32)
    z3 = zpad[idx3].astype(np.float32)
    y4 = y.reshape(128, S).copy()
    z4 = z.reshape(128, S).copy()

    in_maps = []
    for c in range(n_cores):
        rows = slice(c * rpc, (c + 1) * rpc)
        aT15 = np.ascontiguousarray(
            A15[rows].reshape(T, 128, 15).transpose(0, 2, 1)
        )                                            # [T, 15, 128]
        bTs = np.stack([
            np.ascontiguousarray(B15p[c * rpc + 128 * t:
                                      c * rpc + 128 * t + slab].T)
            for t in range(T)
        ])                                           # [T, 15, slab]
        flag = np.full((128, 1), 1.0 if c == 0 else 0.0, np.float32)
        in_maps.append({
            "aT15": aT15, "bT15": bT15, "bTs": bTs,
            "ltmask": ltmask, "ident": ident,
            "y3": y3, "z3": z3, "y4": y4, "z4": z4, "flag": flag,
        })
    return in_maps


def combine_fast(outs, n=N_TOTAL):
    """Returns (total, band_overflowed)."""
    s = np.zeros(8, np.float64)
    overflow = 0.0
    for o in outs:
        a = np.asarray(o, np.float64)
        s += a.sum(axis=0)
        overflow = max(overflow, a[:, 6].max())
    SD, SR, SMALL = s[0], s[1], s[5]
    SRRS, BLK, TR = -s[2], -s[3], -s[4]   # slab r computed negated on device
    masked = SRRS - (BLK - TR) / 2.0
    total = SMALL + SD / 1e4 + SR - masked
    return np.array(total, dtype=np.float32), overflow > 0.0


# ---------------------------------------------------------------------------
# Fallback: exact full-width program for inputs whose prefix runs exceed
# the band. Full-row scan; lower-triangle sums via symmetry of r.
#   intersect = SR - SRR + (SR - N)/2
# ---------------------------------------------------------------------------

def build_nc_full(n=N_TOTAL, row_tiles=ROW_TILES, chunk=512):
    import concourse.bass as bass
    import concourse.mybir as mybir
    from concourse.tile import TileContext

    f32 = mybir.dt.float32
    bf16 = mybir.dt.bfloat16
    AF = mybir.ActivationFunctionType
    ALU = mybir.AluOpType
    AX = mybir.AxisListType

    T = row_tiles
    n_chunks = n // chunk
    OFF = 128 * (T - 1)
    W = n + OFF
    md2 = MIN_DIST * MIN_DIST
    S = n // 128

    nc = bass.Bass()

    aT_d = nc.dram_tensor("aT", [T, 5, 128], f32, kind="ExternalInput")
    bT_d = nc.dram_tensor("bT", [5, n], f32, kind="ExternalInput")
    rowv0_d = nc.dram_tensor("rowv0", [128, 1], f32, kind="ExternalInput")
    out_d = nc.dram_tensor("out", [128, 4], f32, kind="ExternalOutput")

    with TileContext(nc) as tc:
        with (
            tc.tile_pool(name="big", bufs=1) as big,
            tc.tile_pool(name="cst", bufs=1) as cst,
            tc.tile_pool(name="acc", bufs=1) as acc,
            tc.tile_pool(name="wts", bufs=2) as wts,
            tc.tile_pool(name="psum", bufs=8, space="PSUM") as psum,
        ):
            bT = cst.tile([5, n], f32)
            nc.sync.dma_start(out=bT[:], in_=bT_d[:])
            rowv0 = cst.tile([128, 1], f32)
            nc.sync.dma_start(out=rowv0[:], in_=rowv0_d[:])

            iotaM = big.tile([128, W], f32, tag="r")
            nc.gpsimd.iota(
                iotaM[:], pattern=[[1, W]], base=0, channel_multiplier=0,
                allow_small_or_imprecise_dtypes=True,
            )
            ltM = cst.tile([128, W], bf16)
            nc.gpsimd.tensor_scalar(
                out=ltM[:], in0=iotaM[:], scalar1=float(OFF),
                scalar2=rowv0[:], op0=ALU.subtract, op1=ALU.is_lt,
            )

            sd_slots = acc.tile([128, T], f32)
            sr_slots = acc.tile([128, T], f32)
            srr_slots = acc.tile([128, T], f32)
            out_sb = acc.tile([128, 4], f32)

            _small_losses(nc, acc, mybir, out_sb[:, 3:4], S)

            for t in range(T):
                aT_t = wts.tile([5, 128], f32, tag="aT")
                nc.sync.dma_start(out=aT_t[:], in_=aT_d[t])

                d2c = big.tile([128, n], f32, tag="d2c")
                for k in range(n_chunks):
                    pk = psum.tile([128, chunk], f32, tag="pk")
                    nc.tensor.matmul(
                        pk[:], aT_t[:], bT[:, k * chunk : (k + 1) * chunk],
                        start=True, stop=True,
                    )
                    nc.scalar.activation(
                        d2c[:, k * chunk : (k + 1) * chunk], pk[:], AF.Relu
                    )

                dist = big.tile([128, n], f32, tag="dist")
                nc.scalar.activation(dist[:], d2c[:], AF.Sqrt,
                                     accum_out=sd_slots[:, t : t + 1])
                r = big.tile([128, n], f32, tag="r")
                nc.scalar.activation(r[:], dist[:], AF.Relu, bias=1.0,
                                     scale=-1.0,
                                     accum_out=sr_slots[:, t : t + 1])

                cond = big.tile([128, n], bf16, tag="cond")
                s0 = OFF - 128 * t
                nc.vector.scalar_tensor_tensor(
                    out=cond[:], in0=d2c[:], scalar=md2,
                    in1=ltM[:, s0 : s0 + n],
                    op0=ALU.is_le, op1=ALU.logical_or,
                )
                run = big.tile([128, n], bf16, tag="run")
                nc.vector.tensor_tensor_scan(
                    out=run[:], data0=cond[:], data1=cond[:], initial=1.0,
                    op0=ALU.mult, op1=ALU.min,
                )
                nc.vector.scalar_tensor_tensor(
                    out=cond[:], in0=r[:], scalar=1.0, in1=run[:],
                    op0=ALU.mult, op1=ALU.mult,
                    accum_out=srr_slots[:, t : t + 1],
                )

            nc.vector.tensor_reduce(out=out_sb[:, 0:1], in_=sd_slots[:],
                                    axis=AX.X, op=ALU.add)
            nc.vector.tensor_reduce(out=out_sb[:, 1:2], in_=sr_slots[:],
                                    axis=AX.X, op=ALU.add)
            nc.vector.tensor_reduce(out=out_sb[:, 2:3], in_=srr_slots[:],
                                    axis=AX.X, op=ALU.add)
            nc.sync.dma_start(out=out_d[:], in_=out_sb[:])

    return nc


def make_core_inputs_full(x, y, z, n=N_TOTAL, n_cores=N_CORES):
    x = np.asarray(x, np.float32)
    y = np.asarray(y, np.float32)
    z = np.asarray(z, np.float32)
    rpc = n // n_cores
    T = rpc // 128
    S = n // 128

    sq = x * x + y * y + z * z
    ones = np.ones(n, np.float32)
    B = np.stack([-2.0 * x, -2.0 * y, -2.0 * z, ones, sq]).astype(np.float32)

    def _pad_extrap(v):
        two = np.float32(2.0)
        p0 = np.float32(two * v[-1] - v[-2])
        p1 = -np.float32(v[-1] - two * p0)
        return np.concatenate([v, [p0, p1]]).astype(np.float32)

    ypad = _pad_extrap(y)
    zpad = _pad_extrap(z)
    idx3 = S * np.arange(128)[:, None] + np.arange(S + 2)[None, :]
    y3 = ypad[idx3].astype(np.float32)
    z3 = zpad[idx3].astype(np.float32)
    y4 = y.reshape(128, S).copy()
    z4 = z.reshape(128, S).copy()

    in_maps = []
    for c in range(n_cores):
        sl = slice(c * rpc, (c + 1) * rpc)
        A = np.stack([x[sl], y[sl], z[sl], sq[sl], ones[sl]], axis=1)
        aT = np.ascontiguousarray(
            A.reshape(T, 128, 5).transpose(0, 2, 1)
        ).astype(np.float32)
        rowv0 = (c * rpc + np.arange(128, dtype=np.float32)).reshape(128, 1)
        flag = np.full((128, 1), 1.0 if c == 0 else 0.0, np.float32)
        in_maps.append({
            "aT": aT, "bT": B, "rowv0": rowv0,
            "y3": y3, "z3": z3, "y4": y4, "z4": z4, "flag": flag,
        })
    return in_maps


def combine_full(outs, n=N_TOTAL):
    s = np.zeros(4, np.float64)
    for o in outs:
        s += np.asarray(o, np.float64).sum(axis=0)
    SD, SR, SRR, SMALL = s
    intersect = SR - SRR + (SR - float(n)) / 2.0
    total = SMALL + SD / 1e4 + intersect
    return np.array(total, dtype=np.float32)


def kernel(x, y, z):
    from concourse.bass_utils import run_bass_kernel_spmd

    if "fast" not in _CACHE:
        nc = build_nc_fast()
        _split_excess_waits(nc)
        _CACHE["fast"] = nc

    in_maps = make_core_inputs_fast(x, y, z)
    res = run_bass_kernel_spmd(_CACHE["fast"], in_maps, list(range(N_CORES)))
    outs = [res.results[c]["out"] for c in range(N_CORES)]
    total, overflow = combine_fast(outs)
    if not overflow:
        return total

    # a prefix run reached the band edge: rerun with the exact full-width
    # program (arbitrary-input correct, slower)
    if "full" not in _CACHE:
        nc = build_nc_full()
        _split_excess_waits(nc)
        _CACHE["full"] = nc
    in_maps = make_core_inputs_full(x, y, z)
    res = run_bass_kernel_spmd(_CACHE["full"], in_maps, list(range(N_CORES)))
    outs = [res.results[c]["out"] for c in range(N_CORES)]
    return combine_full(outs)


# revision 18
# speedup vs baseline: 1.3853x; 1.1748x over previous
"""Trainium2 Bass kernel for nn_NeuronTurboModel_40819369181759 (nms_detection).

Computes, for N=8192 points on a 3D curve:
    total = sum(puffer(p2d)) + sum(cdist(p3d,p3d))/1e4
          + sum(relu(min_dist - masked_dist)) + 10*sum(second_diff(p2d)^2)
where masked_dist overwrites, per row i, the contiguous run of entries
j >= i with dist <= min_dist (prefix run from the diagonal).

Sharding: rows of the [N,N] distance matrix are split across 8 cores
(1024 rows each); the point set is replicated. Host combines per-core
per-partition partial sums.

Fast path (build_nc_fast):
  - d2 via one K=15 bf16 matmul per chunk: fp32 operands are split hi/lo
    into bf16 (x = h + l); d2 = Ah.Bh + Ah.Bl + Al.Bh stacked into K=15
    (bf16 products are exact; |dropped l.l| ~ 1e-7). PE runs at full bf16
    rate instead of the 4x fp32 penalty.
  - full row [128, 8192]: clamp d2 (relu; matmul cancellation noise can be
    ~ -1e-4 and HW sqrt(neg) = NaN), sqrt -> dist bf16 (accum -> SD).
    SR = sum relu(1-dist) is recovered without an extra pass from
    SR = rows*N - sum(min(dist, 1)) -- one DVE tensor_scalar with accum.
  - the prefix-run mask only needs a diagonal band: run lengths are
    bounded by the band width for curve-like inputs. Each 128-row tile
    processes a [128, 1024] slab starting at its first row (host-sliced
    per core, so SPMD-safe; columns past N are padded with a far point
    that breaks every run). In-slab j<i and j==i masks are compile-time
    static (u<p / u==p), host-provided.
  - per-row-tile slab work: its own small matmul -> clamp -> sqrt -> r;
    cond = (d2 <= 1) | (u < p); run = prefix-product scan; partial sums
    SRRS = sum r*run, BLK = sum r[:, 0:128] (the symmetric diagonal
    block), TR = sum diag(r). Host:
      masked = SRRS - (BLK - TR)/2
      total = SMALL + SD/1e4 + SR - masked
  - validity: if any run reaches the slab end (run[:, -1] != 0), the band
    assumption failed; host falls back to the exact full-width program
    (build_nc_full, the v1 kernel) -- correct for arbitrary inputs.
"""

import numpy as np

N_TOTAL = 8192
N_CORES = 8
ROWS_PER_CORE = N_TOTAL // N_CORES          # 1024
ROW_TILES = ROWS_PER_CORE // 128            # 8
SLAB = 1024                                 # band width (2 PSUM banks)
MIN_DIST = 1.0                              # RADIUS(0.5) * 2 * scale(1.0)

_CACHE = {}


def _split_excess_waits(nc, max_waits=1):
    """Split >max_waits sem waits per instruction onto preceding same-engine NOPs.

    The bundled walrus rejects instructions with several sync waits
    ("Too many sync wait commands"), while Tile's wait assignment can put one
    wait per producer proc on a single instruction. Waits execute in stream
    order on the owning engine, so hoisting the excess onto NOPs immediately
    before the instruction is semantics-preserving.
    """
    import concourse.mybir as mybir

    ctr = [0]
    for f in nc.m.functions:
        for b in f.blocks:
            changed = False
            new = []
            for inst in b.instructions:
                si = inst.sync_info
                if si is not None and len(si.on_wait) > max_waits:
                    waits = list(si.on_wait)
                    ups = list(si.on_update)
                    chunks = [
                        waits[i : i + max_waits]
                        for i in range(0, len(waits), max_waits)
                    ]
                    for ch in chunks[:-1]:
                        ctr[0] += 1
                        nop = mybir.InstNoOp(
                            name=f"I-waitsplit-{ctr[0]}", ins=[], outs=[]
                        )
                        nop.engine = inst.engine
                        nop.sync_info = mybir.SyncInfo(on_wait=ch, on_update=[])
                        new.append(nop)
                    inst.sync_info = mybir.SyncInfo(
                        on_wait=chunks[-1], on_update=ups
                    )
                    changed = True
                new.append(inst)
            if changed:
                b.instructions = new


def _small_losses(nc, pool, mybir, out_col, S):
    """Puffer + curvature losses on [128, S] layouts; ~1% of runtime."""
    f32 = mybir.dt.float32
    AF = mybir.ActivationFunctionType
    ALU = mybir.AluOpType
    AX = mybir.AxisListType

    y3_d = nc.dram_tensor("y3", [128, S + 2], f32, kind="ExternalInput")
    z3_d = nc.dram_tensor("z3", [128, S + 2], f32, kind="ExternalInput")
    y4_d = nc.dram_tensor("y4", [128, S], f32, kind="ExternalInput")
    z4_d = nc.dram_tensor("z4", [128, S], f32, kind="ExternalInput")
    flag_d = nc.dram_tensor("flag", [128, 1], f32, kind="ExternalInput")

    flag = pool.tile([128, 1], f32)
    nc.sync.dma_start(out=flag[:], in_=flag_d[:])
    y3 = pool.tile([128, S + 2], f32)
    z3 = pool.tile([128, S + 2], f32)
    y4 = pool.tile([128, S], f32)
    z4 = pool.tile([128, S], f32)
    nc.sync.dma_start(out=y3[:], in_=y3_d[:])
    nc.sync.dma_start(out=z3[:], in_=z3_d[:])
    nc.sync.dma_start(out=y4[:], in_=y4_d[:])
    nc.sync.dma_start(out=z4[:], in_=z4_d[:])

    fs = pool.tile([128, 4], f32)
    ns = pool.tile([128, 2], f32)
    scr = pool.tile([128, S], f32)
    t1 = pool.tile([128, S], f32)
    neg1 = pool.tile([128, 1], f32)
    nc.vector.memset(neg1[:], -1.0)

    # fixed (puffer): relu(v - 1) + relu(-1 - v) for v in {y, z}
    nc.scalar.activation(scr[:], y4[:], AF.Relu, bias=neg1[:], scale=1.0,
                         accum_out=fs[:, 0:1])
    nc.scalar.activation(scr[:], y4[:], AF.Relu, bias=neg1[:], scale=-1.0,
                         accum_out=fs[:, 1:2])
    nc.scalar.activation(scr[:], z4[:], AF.Relu, bias=neg1[:], scale=1.0,
                         accum_out=fs[:, 2:3])
    nc.scalar.activation(scr[:], z4[:], AF.Relu, bias=neg1[:], scale=-1.0,
                         accum_out=fs[:, 3:4])

    # noise: sum (v[k] - 2 v[k+1] + v[k+2])^2; host pads v3 by linear
    # extrapolation so the two out-of-range terms are exactly zero
    for i, v3 in enumerate((y3, z3)):
        nc.vector.scalar_tensor_tensor(
            out=t1[:], in0=v3[:, 1 : S + 1], scalar=-2.0, in1=v3[:, 0:S],
            op0=ALU.mult, op1=ALU.add,
        )
        nc.vector.tensor_tensor(t1[:], t1[:], v3[:, 2 : S + 2], ALU.add)
        nc.scalar.activation(scr[:], t1[:], AF.Square,
                             accum_out=ns[:, i : i + 1])

    fsum = pool.tile([128, 1], f32)
    nsum = pool.tile([128, 1], f32)
    nc.vector.tensor_reduce(out=fsum[:], in_=fs[:], axis=AX.X, op=ALU.add)
    nc.vector.tensor_reduce(out=nsum[:], in_=ns[:], axis=AX.X, op=ALU.add)
    nc.vector.scalar_tensor_tensor(
        out=fsum[:], in0=nsum[:], scalar=10.0, in1=fsum[:],
        op0=ALU.mult, op1=ALU.add,
    )
    nc.vector.tensor_tensor(out_col, fsum[:], flag[:], ALU.mult)


def build_nc_fast(n=N_TOTAL, row_tiles=ROW_TILES, slab=SLAB):
    """Band-limited fast program (same SPMD program on all 8 cores)."""
    import concourse.bass as bass
    import concourse.mybir as mybir
    from concourse.tile import TileContext

    f32 = mybir.dt.float32
    bf16 = mybir.dt.bfloat16
    AF = mybir.ActivationFunctionType
    ALU = mybir.AluOpType
    AX = mybir.AxisListType

    T = row_tiles
    S = n // 128
    md2 = MIN_DIST * MIN_DIST
    PW = 1024                       # psum tile width (2 banks)
    NQ = n // PW                    # full-row psum tiles per row tile

    nc = bass.Bass()

    aT_d = nc.dram_tensor("aT15", [T, 15, 128], bf16, kind="ExternalInput")
    bT_d = nc.dram_tensor("bT15", [15, n], bf16, kind="ExternalInput")
    bs_d = nc.dram_tensor("bTs", [T, 15, slab], bf16, kind="ExternalInput")
    lt_d = nc.dram_tensor("ltmask", [128, slab], bf16, kind="ExternalInput")
    id_d = nc.dram_tensor("ident", [128, 128], bf16, kind="ExternalInput")
    out_d = nc.dram_tensor("out", [128, 8], f32, kind="ExternalOutput")

    with TileContext(nc) as tc:
        with (
            tc.tile_pool(name="big", bufs=1) as big,
            tc.tile_pool(name="scrp", bufs=2) as scrp,
            tc.tile_pool(name="slb", bufs=2) as slb,
            tc.tile_pool(name="cst", bufs=1) as cst,
            tc.tile_pool(name="acc", bufs=1) as acc,
            tc.tile_pool(name="wts", bufs=2) as wts,
            tc.tile_pool(name="pf", bufs=2, space="PSUM") as pf,
            tc.tile_pool(name="psl", bufs=2, space="PSUM") as psl,
        ):
            bT = cst.tile([15, n], bf16)
            nc.sync.dma_start(out=bT[:], in_=bT_d[:])
            ltm = cst.tile([128, slab], bf16)
            nc.sync.dma_start(out=ltm[:], in_=lt_d[:])
            ident = cst.tile([128, 128], bf16)
            nc.sync.dma_start(out=ident[:], in_=id_d[:])

            sd_slots = acc.tile([128, T], f32)
            srm_slots = acc.tile([128, T], f32)
            srr_slots = acc.tile([128, T], f32)
            blk_slots = acc.tile([128, T], f32)
            tr_slots = acc.tile([128, T], f32)
            re_slots = acc.tile([128, T], f32)
            out_sb = acc.tile([128, 8], f32)

            _small_losses(nc, acc, mybir, out_sb[:, 5:6], S)

            for t in range(T):
                aTt = wts.tile([15, 128], bf16, tag="aTt")
                nc.sync.dma_start(out=aTt[:], in_=aT_d[t])
                bst = wts.tile([15, slab], bf16, tag="bst")
                nc.sync.dma_start(out=bst[:], in_=bs_d[t])

                # ---- full row: d2 -> dist -> SD accum; relu(1-dist) -> SR.
                # d2/dist stay f32: SR needs ~5e-6 relative accuracy, which
                # bf16-rounded d2/dist break.
                dist = big.tile([128, n], f32, tag="dist")
                scr = scrp.tile([128, n], f32, tag="scr")
                sink = big.tile([128, n], bf16, tag="sink")
                for q in range(NQ):
                    pk = pf.tile([128, PW], f32, tag="pk")
                    for h in range(PW // 512):
                        c0 = q * PW + h * 512
                        nc.tensor.matmul(
                            pk[:, h * 512 : (h + 1) * 512],
                            aTt[:], bT[:, c0 : c0 + 512],
                            start=True, stop=True,
                        )
                    # clamp cancellation noise on DVE (ACT is the busier
                    # engine: sqrt + r-pass)
                    nc.vector.tensor_scalar(
                        out=scr[:, q * PW : (q + 1) * PW], in0=pk[:],
                        scalar1=0.0, scalar2=None, op0=ALU.max,
                    )
                nc.scalar.activation(dist[:], scr[:], AF.Sqrt,
                                     accum_out=sd_slots[:, t : t + 1])
                # SR partial: ACT relu(1 - dist) with accum; output discarded
                nc.scalar.activation(sink[:], dist[:], AF.Relu,
                                     bias=1.0, scale=-1.0,
                                     accum_out=srm_slots[:, t : t + 1])

                # ---- slab: band-limited prefix-run mask ----
                ps = psl.tile([128, slab], f32, tag="ps")
                for h in range(slab // 512):
                    nc.tensor.matmul(
                        ps[:, h * 512 : (h + 1) * 512],
                        aTt[:], bst[:, h * 512 : (h + 1) * 512],
                        start=True, stop=True,
                    )
                d2s = slb.tile([128, slab], f32, tag="d2s")
                nc.vector.tensor_scalar(out=d2s[:], in0=ps[:],
                                        scalar1=0.0, scalar2=None, op0=ALU.max)
                dss = slb.tile([128, slab], f32, tag="dss")
                nc.scalar.activation(dss[:], d2s[:], AF.Sqrt)
                # rs = -relu(1 - dist_s) = min(dist_s - 1, 0); the host
                # negates the SRRS/BLK/TR partials it feeds
                rs = slb.tile([128, slab], f32, tag="rs")
                nc.vector.tensor_scalar(out=rs[:], in0=dss[:],
                                        scalar1=1.0, scalar2=0.0,
                                        op0=ALU.subtract, op1=ALU.min)

                cond = slb.tile([128, slab], bf16, tag="cond")
                nc.vector.scalar_tensor_tensor(
                    out=cond[:], in0=d2s[:], scalar=md2, in1=ltm[:],
                    op0=ALU.is_le, op1=ALU.logical_or,
                )
                run = slb.tile([128, slab], bf16, tag="run")
                nc.vector.tensor_tensor_scan(
                    out=run[:], data0=cond[:], data1=cond[:], initial=1.0,
                    op0=ALU.mult, op1=ALU.min,
                )
                nc.vector.scalar_tensor_tensor(
                    out=cond[:], in0=rs[:], scalar=1.0, in1=run[:],
                    op0=ALU.mult, op1=ALU.mult,
                    accum_out=srr_slots[:, t : t + 1],
                )
                nc.vector.tensor_scalar(
                    out=cond[:, 0:128], in0=rs[:, 0:128], scalar1=1.0,
                    scalar2=0.0, op0=ALU.mult, op1=ALU.add,
                    accum_out=blk_slots[:, t : t + 1],
                )
                nc.vector.scalar_tensor_tensor(
                    out=cond[:, 0:128], in0=rs[:, 0:128], scalar=1.0,
                    in1=ident[:], op0=ALU.mult, op1=ALU.mult,
                    accum_out=tr_slots[:, t : t + 1],
                )
                nc.vector.tensor_copy(
                    re_slots[:, t : t + 1], run[:, slab - 1 : slab]
                )

            nc.vector.tensor_reduce(out=out_sb[:, 0:1], in_=sd_slots[:],
                                    axis=AX.X, op=ALU.add)
            nc.vector.tensor_reduce(out=out_sb[:, 1:2], in_=srm_slots[:],
                                    axis=AX.X, op=ALU.add)
            nc.vector.tensor_reduce(out=out_sb[:, 2:3], in_=srr_slots[:],
                                    axis=AX.X, op=ALU.add)
            nc.vector.tensor_reduce(out=out_sb[:, 3:4], in_=blk_slots[:],
                                    axis=AX.X, op=ALU.add)
            nc.vector.tensor_reduce(out=out_sb[:, 4:5], in_=tr_slots[:],
                                    axis=AX.X, op=ALU.add)
            nc.vector.tensor_reduce(out=out_sb[:, 6:7], in_=re_slots[:],
                                    axis=AX.X, op=ALU.max)
            nc.vector.memset(out_sb[:, 7:8], 0.0)
            nc.sync.dma_start(out=out_d[:], in_=out_sb[:])

    return nc


def _bf16_split(a):
    import ml_dtypes
    bf = ml_dtypes.bfloat16
    h = a.astype(bf)
    l = (a - h.astype(np.float32)).astype(bf)
    return h, l


def make_core_inputs_fast(x, y, z, n=N_TOTAL, n_cores=N_CORES, slab=SLAB):
    import ml_dtypes
    bf = ml_dtypes.bfloat16
    x = np.asarray(x, np.float32)
    y = np.asarray(y, np.float32)
    z = np.asarray(z, np.float32)
    rpc = n // n_cores
    T = rpc // 128
    S = n // 128

    sq = x * x + y * y + z * z
    ones = np.ones(n, np.float32)
    A = np.stack([x, y, z, sq, ones], 1)             # [n, 5]
    B = np.stack([-2 * x, -2 * y, -2 * z, ones, sq], 1)
    Ah, Al = _bf16_split(A)
    Bh, Bl = _bf16_split(B)
    A15 = np.concatenate([Ah, Ah, Al], 1)            # [n, 15] bf16
    B15 = np.concatenate([Bh, Bl, Bh], 1)
    bT15 = np.ascontiguousarray(B15.T)               # [15, n]

    # pad columns past n with a far point (breaks runs, contributes r=0)
    far = np.array([1e3, 0.0, 0.0], np.float32)
    fsq = np.float32(far @ far)
    Bf = np.array([-2 * far[0], -2 * far[1], -2 * far[2], 1.0, fsq],
                  np.float32)
    Bfh, Bfl = _bf16_split(Bf)
    Bf15 = np.concatenate([Bfh, Bfl, Bfh]).astype(bf)  # [15]
    B15p = np.concatenate([B15, np.broadcast_to(Bf15, (slab, 15))], 0)

    ltmask = (np.arange(slab)[None, :] < np.arange(128)[:, None]).astype(bf)
    ident = np.eye(128).astype(bf)

    def _pad_extrap(v):
        two = np.float32(2.0)
        p0 = np.float32(two * v[-1] - v[-2])
        p1 = -np.float32(v[-1] - two * p0)
        return np.concatenate([v, [p0, p1]]).astype(np.float32)

    ypad = _pad_extrap(y)
    zpad = _pad_extrap(z)
    idx3 = S * np.arange(128)[:, None] + np.arange(S + 2)[None, :]
    y3 = ypad[idx3].astype(np.float32)
    z3 = zpad[idx3].astype(np.float32)
    y4 = y.reshape(128, S).copy()
    z4 = z.reshape(128, S).copy()

    in_maps = []
    for c in range(n_cores):
        rows = slice(c * rpc, (c + 1) * rpc)
        aT15 = np.ascontiguousarray(
            A15[rows].reshape(T, 128, 15).transpose(0, 2, 1)
        )                                            # [T, 15, 128]
        bTs = np.stack([
            np.ascontiguousarray(B15p[c * rpc + 128 * t:
                                      c * rpc + 128 * t + slab].T)
            for t in range(T)
        ])                                           # [T, 15, slab]
        flag = np.full((128, 1), 1.0 if c == 0 else 0.0, np.float32)
        in_maps.append({
            "aT15": aT15, "bT15": bT15, "bTs": bTs,
            "ltmask": ltmask, "ident": ident,
            "y3": y3, "z3": z3, "y4": y4, "z4": z4, "flag": flag,
        })
    return in_maps


def combine_fast(outs, n=N_TOTAL):
    """Returns (total, band_overflowed)."""
    s = np.zeros(8, np.float64)
    overflow = 0.0
    for o in outs:
        a = np.asarray(o, np.float64)
        s += a.sum(axis=0)
        overflow = max(overflow, a[:, 6].max())
    SD, SR, SMALL = s[0], s[1], s[5]
    SRRS, BLK, TR = -s[2], -s[3], -s[4]   # slab r computed negated on device
    masked = SRRS - (BLK - TR) / 2.0
    total = SMALL + SD / 1e4 + SR - masked
    return np.array(total, dtype=np.float32), overflow > 0.0


# ---------------------------------------------------------------------------
# Fallback: exact full-width program for inputs whose prefix runs exceed
# the band. Full-row scan; lower-triangle sums via symmetry of r.
#   intersect = SR - SRR + (SR - N)/2
# ---------------------------------------------------------------------------

def build_nc_full(n=N_TOTAL, row_tiles=ROW_TILES, chunk=512):
    import concourse.bass as bass
    import concourse.mybir as mybir
    from concourse.tile import TileContext

    f32 = mybir.dt.float32
    bf16 = mybir.dt.bfloat16
    AF = mybir.ActivationFunctionType
    ALU = mybir.AluOpType
    AX = mybir.AxisListType

    T = row_tiles
    n_chunks = n // chunk
    OFF = 128 * (T - 1)
    W = n + OFF
    md2 = MIN_DIST * MIN_DIST
    S = n // 128

    nc = bass.Bass()

    aT_d = nc.dram_tensor("aT", [T, 5, 128], f32, kind="ExternalInput")
    bT_d = nc.dram_tensor("bT", [5, n], f32, kind="ExternalInput")
    rowv0_d = nc.dram_tensor("rowv0", [128, 1], f32, kind="ExternalInput")
    out_d = nc.dram_tensor("out", [128, 4], f32, kind="ExternalOutput")

    with TileContext(nc) as tc:
        with (
            tc.tile_pool(name="big", bufs=1) as big,
            tc.tile_pool(name="cst", bufs=1) as cst,
            tc.tile_pool(name="acc", bufs=1) as acc,
            tc.tile_pool(name="wts", bufs=2) as wts,
            tc.tile_pool(name="psum", bufs=8, space="PSUM") as psum,
        ):
            bT = cst.tile([5, n], f32)
            nc.sync.dma_start(out=bT[:], in_=bT_d[:])
            rowv0 = cst.tile([128, 1], f32)
            nc.sync.dma_start(out=rowv0[:], in_=rowv0_d[:])

            iotaM = big.tile([128, W], f32, tag="r")
            nc.gpsimd.iota(
                iotaM[:], pattern=[[1, W]], base=0, channel_multiplier=0,
                allow_small_or_imprecise_dtypes=True,
            )
            ltM = cst.tile([128, W], bf16)
            nc.gpsimd.tensor_scalar(
                out=ltM[:], in0=iotaM[:], scalar1=float(OFF),
                scalar2=rowv0[:], op0=ALU.subtract, op1=ALU.is_lt,
            )

            sd_slots = acc.tile([128, T], f32)
            sr_slots = acc.tile([128, T], f32)
            srr_slots = acc.tile([128, T], f32)
            out_sb = acc.tile([128, 4], f32)

            _small_losses(nc, acc, mybir, out_sb[:, 3:4], S)

            for t in range(T):
                aT_t = wts.tile([5, 128], f32, tag="aT")
                nc.sync.dma_start(out=aT_t[:], in_=aT_d[t])

                d2c = big.tile([128, n], f32, tag="d2c")
                for k in range(n_chunks):
                    pk = psum.tile([128, chunk], f32, tag="pk")
                    nc.tensor.matmul(
                        pk[:], aT_t[:], bT[:, k * chunk : (k + 1) * chunk],
                        start=True, stop=True,
                    )
                    nc.scalar.activation(
                        d2c[:, k * chunk : (k + 1) * chunk], pk[:], AF.Relu
                    )

                dist = big.tile([128, n], f32, tag="dist")
                nc.scalar.activation(dist[:], d2c[:], AF.Sqrt,
                                     accum_out=sd_slots[:, t : t + 1])
                r = big.tile([128, n], f32, tag="r")
                nc.scalar.activation(r[:], dist[:], AF.Relu, bias=1.0,
                                     scale=-1.0,
                                     accum_out=sr_slots[:, t : t + 1])

                cond = big.tile([128, n], bf16, tag="cond")
                s0 = OFF - 128 * t
                nc.vector.scalar_tensor_tensor(
                    out=cond[:], in0=d2c[:], scalar=md2,
                    in1=ltM[:, s0 : s0 + n],
                    op0=ALU.is_le, op1=ALU.logical_or,
                )
                run = big.tile([128, n], bf16, tag="run")
                nc.vector.tensor_tensor_scan(
                    out=run[:], data0=cond[:], data1=cond[:], initial=1.0,
                    op0=ALU.mult, op1=ALU.min,
                )
                nc.vector.scalar_tensor_tensor(
                    out=cond[:], in0=r[:], scalar=1.0, in1=run[:],
                    op0=ALU.mult, op1=ALU.mult,
                    accum_out=srr_slots[:, t : t + 1],
                )

            nc.vector.tensor_reduce(out=out_sb[:, 0:1], in_=sd_slots[:],
                                    axis=AX.X, op=ALU.add)
            nc.vector.tensor_reduce(out=out_sb[:, 1:2], in_=sr_slots[:],
                                    axis=AX.X, op=ALU.add)
            nc.vector.tensor_reduce(out=out_sb[:, 2:3], in_=srr_slots[:],
                                    axis=AX.X, op=ALU.add)
            nc.sync.dma_start(out=out_d[:], in_=out_sb[:])

    return nc


def make_core_inputs_full(x, y, z, n=N_TOTAL, n_cores=N_CORES):
    x = np.asarray(x, np.float32)
    y = np.asarray(y, np.float32)
    z = np.asarray(z, np.float32)
    rpc = n // n_cores
    T = rpc // 128
    S = n // 128

    sq = x * x + y * y + z * z
    ones = np.ones(n, np.float32)
    B = np.stack([-2.0 * x, -2.0 * y, -2.0 * z, ones, sq]).astype(np.float32)

    def _pad_extrap(v):
        two = np.float32(2.0)
        p0 = np.float32(two * v[-1] - v[-2])
        p1 = -np.float32(v[-1] - two * p0)
        return np.concatenate([v, [p0, p1]]).astype(np.float32)

    ypad = _pad_extrap(y)
    zpad = _pad_extrap(z)
    idx3 = S * np.arange(128)[:, None] + np.arange(S + 2)[None, :]
    y3 = ypad[idx3].astype(np.float32)
    z3 = zpad[idx3].astype(np.float32)
    y4 = y.reshape(128, S).copy()
    z4 = z.reshape(128, S).copy()

    in_maps = []
    for c in range(n_cores):
        sl = slice(c * rpc, (c + 1) * rpc)
        A = np.stack([x[sl], y[sl], z[sl], sq[sl], ones[sl]], axis=1)
        aT = np.ascontiguousarray(
            A.reshape(T, 128, 5).transpose(0, 2, 1)
        ).astype(np.float32)
        rowv0 = (c * rpc + np.arange(128, dtype=np.float32)).reshape(128, 1)
        flag = np.full((128, 1), 1.0 if c == 0 else 0.0, np.float32)
        in_maps.append({
            "aT": aT, "bT": B, "rowv0": rowv0,
            "y3": y3, "z3": z3, "y4": y4, "z4": z4, "flag": flag,
        })
    return in_maps


def combine_full(outs, n=N_TOTAL):
    s = np.zeros(4, np.float64)
    for o in outs:
        s += np.asarray(o, np.float64).sum(axis=0)
    SD, SR, SRR, SMALL = s
    intersect = SR - SRR + (SR - float(n)) / 2.0
    total = SMALL + SD / 1e4 + intersect
    return np.array(total, dtype=np.float32)


def kernel(x, y, z):
    from concourse.bass_utils import run_bass_kernel_spmd

    if "fast" not in _CACHE:
        nc = build_nc_fast()
        _split_excess_waits(nc)
        _CACHE["fast"] = nc

    in_maps = make_core_inputs_fast(x, y, z)
    res = run_bass_kernel_spmd(_CACHE["fast"], in_maps, list(range(N_CORES)))
    outs = [res.results[c]["out"] for c in range(N_CORES)]
    total, overflow = combine_fast(outs)
    if not overflow:
        return total

    # a prefix run reached the band edge: rerun with the exact full-width
    # program (arbitrary-input correct, slower)
    if "full" not in _CACHE:
        nc = build_nc_full()
        _split_excess_waits(nc)
        _CACHE["full"] = nc
    in_maps = make_core_inputs_full(x, y, z)
    res = run_bass_kernel_spmd(_CACHE["full"], in_maps, list(range(N_CORES)))
    outs = [res.results[c]["out"] for c in range(N_CORES)]
    return combine_full(outs)
